# revision 7
# baseline (speedup 1.0000x reference)
"""GCN layer (2-hop SpMM + per-hop Linear/ReLU) on 8 Trainium2 NeuronCores.

Strategy (dst-sharded graph parallel, per sharding hint):
  - Nodes sharded 1250/core; each core owns the edges pointing at its shard.
  - Host sorts edges by dst, packs them into 128-edge chunks per 128-dst
    block, and builds per-chunk one-hot scatter matrices S (S[p,m] = w_e for
    edge p landing on local dst m) plus a dma_gather index list of src ids.
    Both are identical for the two hops (the graph doesn't change).
  - Per hop: dma_gather pulls h[src] rows (bf16) from a full per-core DRAM
    copy into SBUF G tiles [128 edges, 512 feat]; TensorE computes
    psum += S.T @ G per chunk (the scatter-add); ScalarE evicts with the
    per-dst D_norm scale (bf16); HWDGE DMA-transpose builds the feat-major
    copy used by the linear layers.
  - Hop-1 shards are AllGathered (bf16) into every core's DRAM so hop 2 can
    gather from the full h1.
  - Linear stage runs feat-major: out.T[fo, n] = relu(W.T @ hT + b), bias and
    relu fused in one ScalarE activation (bias is per-partition there).
    The [1536, 1250] per-core outputs are concatenated + transposed on host.
"""

import sys

sys.path.insert(0, "/opt/trn_rl_repo")

import numpy as np
import ml_dtypes

import concourse.bass as bass
import concourse.bacc as bacc
import concourse.mybir as mybir
import concourse.tile as tile
from concourse import library_config
from concourse.bass_utils import run_bass_kernel_spmd

N_NODES = 10000
N_EDGES = 160000
D = 512
ORDER = 2
N_CORES = 8
SHARD = N_NODES // N_CORES          # 1250
BLKS = (SHARD + 127) // 128         # 10 dst blocks per core
BLK_SZ = [min(128, SHARD - b * 128) for b in range(BLKS)]  # [128]*9 + [98]
FI = D // 128                       # 4 feat-in chunks
FO = D // 128                       # 4 feat-out tiles
NGRPS = [512, 512, SHARD - 1024]    # node groups for linear stage
BF16 = ml_dtypes.bfloat16


def _split_excess_waits(nc, max_waits=1):
    """This walrus build rejects >1 sync wait per instruction (and any on a
    Drain). Hoist excess SyncWaits onto InstNoOp carriers inserted just
    before, on the same engine — waits execute in program order, so
    semantics are preserved."""
    for fn in nc.m.functions:
        for bb in fn.blocks:
            new = []
            changed = False
            for inst in bb.instructions:
                si = inst.sync_info
                cap = 0 if isinstance(inst, mybir.InstDrain) else max_waits
                if si is not None and len(si.on_wait) > cap:
                    waits = list(si.on_wait)
                    excess = waits[:-cap] if cap else waits
                    keep = waits[-cap:] if cap else []
                    for g in range(0, len(excess), max_waits):
                        nop = mybir.InstNoOp(name=f"{inst.name}-ws{g}", ins=[], outs=[])
                        nop.engine = inst.engine
                        nop.sync_info = mybir.SyncInfo(
                            on_wait=excess[g:g + max_waits], on_update=[])
                        new.append(nop)
                    si.on_wait = keep
                    changed = True
                new.append(inst)
            if changed:
                bb.instructions = new


def _preprocess(features, D_norm, edge_w, W, b, src, dst):
    """Host-side: shard edges by dst owner, sort, chunk, build S / idx."""
    core_of = dst // SHARD
    per_core = []
    for i in range(N_CORES):
        sel = np.nonzero(core_of == i)[0]
        dl = dst[sel] - i * SHARD          # local dst in [0, SHARD)
        order = np.argsort(dl, kind="stable")
        per_core.append((sel[order], dl[order]))

    # chunk counts per block, maxed over cores so the SPMD program is shared
    nchk = np.zeros(BLKS, np.int64)
    for i in range(N_CORES):
        _, dl = per_core[i]
        blk = dl // 128
        cnt = np.bincount(blk, minlength=BLKS)
        nchk = np.maximum(nchk, (cnt + 127) // 128)
    nchk = np.maximum(nchk, 1).astype(np.int64)
    ncht = int(nchk.sum())

    idx_cols = ncht * 8                     # 16 idx per column
    idx_packed = np.zeros((N_CORES, 128, idx_cols), np.int16)
    s_t = np.zeros((N_CORES, 128, ncht, 128), np.float32)
    blk_chunk_off = np.concatenate([[0], np.cumsum(nchk)])

    for i in range(N_CORES):
        eids, dl = per_core[i]
        w = edge_w[eids]
        s = src[eids]
        blk = dl // 128
        m = dl - blk * 128
        # position of each edge within its block
        cnt = np.bincount(blk, minlength=BLKS)
        boff = np.concatenate([[0], np.cumsum(cnt)])[:-1]
        pos_in_blk = np.arange(len(eids)) - boff[blk]
        chunk = blk_chunk_off[blk] + pos_in_blk // 128
        lane = pos_in_blk % 128
        s_t[i, lane, chunk, m] = w
        lin = chunk * 128 + lane            # linear gather slot
        idx_flat = np.zeros(ncht * 128, np.int16)  # pads gather node 0
        idx_flat[lin] = s.astype(np.int16)
        idx_packed[i] = np.tile(idx_flat.reshape(-1, 16).T, (8, 1))

    return nchk, ncht, idx_packed, s_t.astype(BF16), per_core


def _build_program(nchk, ncht, split_waits=True):
    nc = bacc.Bacc("TRN2")
    dt = mybir.dt

    h0_full = nc.declare_dram_parameter("h0_full", [N_NODES, D], dt.bfloat16, isOutput=False)
    h0t_shard = nc.declare_dram_parameter("h0t_shard", [128, FI, SHARD], dt.bfloat16, isOutput=False)
    idx_in = nc.declare_dram_parameter("idx", [128, ncht * 8], dt.int16, isOutput=False)
    s_in = nc.declare_dram_parameter("s", [128, ncht, 128], dt.bfloat16, isOutput=False)
    d_in = nc.declare_dram_parameter("dnorm", [128, BLKS], dt.float32, isOutput=False)
    w_in = nc.declare_dram_parameter("w", [128, ORDER + 1, FI, D], dt.bfloat16, isOutput=False)
    b_in = nc.declare_dram_parameter("bias", [128, ORDER + 1, FO], dt.float32, isOutput=False)
    out_t = nc.declare_dram_parameter("out_t", [(ORDER + 1) * D, SHARD], dt.float32, isOutput=True)

    h1_shard_dram = nc.dram_tensor("h1_shard", [SHARD, D], dt.bfloat16)
    h1_full = nc.dram_tensor("h1_full", [N_NODES, D], dt.bfloat16, addr_space="Shared")

    blk_off = np.concatenate([[0], np.cumsum(nchk)])

    with tile.TileContext(nc) as tc:
        nc.gpsimd.load_library(library_config.mlp)
        with (
            tc.tile_pool(name="const", bufs=1) as const,
            tc.tile_pool(name="gbuf", bufs=3) as gbuf,
            tc.tile_pool(name="evict", bufs=3) as evict,
            tc.tile_pool(name="lin", bufs=3) as lin,
            tc.tile_pool(name="psum", bufs=4, space=bass.MemorySpace.PSUM) as psum,
            tc.tile_pool(name="psw", bufs=4, space=bass.MemorySpace.PSUM) as psw,
        ):
            idx_t = const.tile([128, ncht * 8], dt.int16)
            nc.sync.dma_start(idx_t[:], idx_in[:])
            s_t = const.tile([128, ncht, 128], dt.bfloat16)
            nc.sync.dma_start(s_t[:], s_in[:])
            d_t = const.tile([128, BLKS], dt.float32)
            nc.sync.dma_start(d_t[:], d_in[:])
            w_t = const.tile([128, ORDER + 1, FI, D], dt.bfloat16)
            nc.sync.dma_start(w_t[:], w_in[:])
            b_t = const.tile([128, ORDER + 1, FO], dt.float32)
            nc.sync.dma_start(b_t[:], b_in[:])

            # feat-major hop results; hT[p, f, n] = h[n, f*128+p]
            # free dim padded to BLKS*128 so the last block's transpose fits
            ht = [const.tile([128, FI, BLKS * 128], dt.bfloat16, tag=f"ht{k}",
                             name=f"ht{k}")
                  for k in range(ORDER + 1)]
            nc.sync.dma_start(ht[0][:, :, :SHARD], h0t_shard[:])

            def hop(src_dram, k):
                """One SpMM hop: gather from src_dram, scatter into shard
                blocks, scale by D_norm, emit bf16 node-major blocks +
                feat-major ht[k]."""
                blocks = []
                for bi in range(BLKS):
                    nch = int(nchk[bi])
                    off = int(blk_off[bi])
                    g = gbuf.tile([128, nch, D], dt.bfloat16, tag="g")
                    nc.gpsimd.dma_gather(
                        out_ap=g[:],
                        in_ap=src_dram[:],
                        idxs_ap=idx_t[:, off * 8:(off + nch) * 8],
                        num_idxs=nch * 128,
                        num_idxs_reg=nch * 128,
                        elem_size=D,
                        # >~2000 idxs overflows the single-packet SWDGE ring
                        single_packet=False,
                    )
                    sz = BLK_SZ[bi]
                    # full 128 dst columns: S pad columns are zero, so rows
                    # beyond the block size come out as defined zeros
                    acc = psum.tile([128, D], dt.float32, tag="agg")
                    for c in range(nch):
                        nc.tensor.matmul(
                            acc[:], s_t[:, off + c, :], g[:, c, :],
                            start=(c == 0), stop=(c == nch - 1))
                    hb = evict.tile([128, D], dt.bfloat16, tag="hb")
                    nc.scalar.activation(
                        out=hb[:], in_=acc[:],
                        func=mybir.ActivationFunctionType.Copy,
                        scale=d_t[:, bi:bi + 1])
                    # feat-major transpose into ht[k]
                    nc.sync.dma_start_transpose(
                        ht[k][:, :, bi * 128:bi * 128 + 128], hb[:])
                    blocks.append((hb, sz))
                return blocks

            blocks1 = hop(h0_full, 1)
            for bi, (hb, sz) in enumerate(blocks1):
                nc.sync.dma_start(
                    h1_shard_dram[bi * 128:bi * 128 + sz, :], hb[:sz, :])
            nc.gpsimd.collective_compute(
                "AllGather",
                mybir.AluOpType.bypass,
                replica_groups=[list(range(N_CORES))],
                ins=[h1_shard_dram[:]],
                outs=[h1_full[:]],
            )
            hop(h1_full, 2)

            # Linear stage, feat-major: outT[fo, n] = relu(sum_fi W[fi,fo]^T hT + b)
            for k in range(ORDER + 1):
                for ft in range(FO):
                    for gi, gsz in enumerate(NGRPS):
                        goff = sum(NGRPS[:gi])
                        pw = psw.tile([128, gsz], dt.float32, tag="pw")
                        for fi in range(FI):
                            nc.tensor.matmul(
                                pw[:], w_t[:, k, fi, ft * 128:(ft + 1) * 128],
                                ht[k][:, fi, goff:goff + gsz],
                                start=(fi == 0), stop=(fi == FI - 1))
                        ob = lin.tile([128, gsz], dt.float32, tag="ob")
                        nc.scalar.activation(
                            out=ob[:], in_=pw[:],
                            func=mybir.ActivationFunctionType.Relu,
                            bias=b_t[:, k, ft:ft + 1])
                        nc.sync.dma_start(
                            out_t[k * D + ft * 128:k * D + (ft + 1) * 128,
                                  goff:goff + gsz], ob[:])

    nc.compile()
    if split_waits:
        _split_excess_waits(nc)
    return nc


def kernel(features, D_norm, edge_w, W, b, src, dst, _timing=None):
    features = np.asarray(features, np.float32)
    D_norm = np.asarray(D_norm, np.float32)
    edge_w = np.asarray(edge_w, np.float32)
    W = np.asarray(W, np.float32)
    b = np.asarray(b, np.float32)
    src = np.asarray(src, np.int32)
    dst = np.asarray(dst, np.int32)

    nchk, ncht, idx_packed, s_t, _ = _preprocess(
        features, D_norm, edge_w, W, b, src, dst)
    nc = _build_program(nchk, ncht)

    h0_bf = features.astype(BF16)
    # h0t[p, f, n] = h0[shard+n, f*128+p]
    d_pack = np.zeros((128, BLKS), np.float32)
    w_pack = np.zeros((128, ORDER + 1, FI, D), np.float32)
    for fi in range(FI):
        w_pack[:, :, fi, :] = W[:, fi * 128:(fi + 1) * 128, :].transpose(1, 0, 2)
    b_pack = np.zeros((128, ORDER + 1, FO), np.float32)
    for ft in range(FO):
        b_pack[:, :, ft] = b[:, ft * 128:(ft + 1) * 128].T

    in_maps = []
    for i in range(N_CORES):
        sh = slice(i * SHARD, (i + 1) * SHARD)
        h0t = features[sh].reshape(SHARD, FI, 128).transpose(2, 1, 0)
        dp = d_pack.copy()
        dflat = D_norm[sh, 0]
        for bi in range(BLKS):
            dp[:BLK_SZ[bi], bi] = dflat[bi * 128:bi * 128 + BLK_SZ[bi]]
        in_maps.append({
            "h0_full": h0_bf,
            "h0t_shard": h0t.astype(BF16).copy(),
            "idx": idx_packed[i],
            "s": s_t[i],
            "dnorm": dp,
            "w": w_pack.astype(BF16),
            "bias": b_pack,
        })

    res = run_bass_kernel_spmd(
        nc, in_maps, list(range(N_CORES)),
        trace=bool(_timing is not None))
    if _timing is not None:
        _timing["exec_time_ns"] = res.exec_time_ns

    parts = [np.asarray(res.results[i]["out_t"]) for i in range(N_CORES)]
    out = np.concatenate(parts, axis=1).T          # [N, 3*D]
    return np.ascontiguousarray(out, dtype=np.float32)


# revision 8
# speedup vs baseline: 1.0717x; 1.0717x over previous
"""GCN layer (2-hop SpMM + per-hop Linear/ReLU) on 8 Trainium2 NeuronCores.

Strategy (dst-sharded graph parallel, per sharding hint):
  - Nodes sharded 1250/core; each core owns the edges pointing at its shard.
  - Host sorts edges by dst, packs them into 128-edge chunks per 128-dst
    block, and builds per-chunk one-hot scatter matrices S (S[p,m] = w_e for
    edge p landing on local dst m) plus a dma_gather index list of src ids.
    Both are identical for the two hops (the graph doesn't change).
  - Per hop: dma_gather pulls h[src] rows (bf16) from a full per-core DRAM
    copy into SBUF G tiles [128 edges, 512 feat]; TensorE computes
    psum += S.T @ G per chunk (the scatter-add); ScalarE evicts with the
    per-dst D_norm scale (bf16); HWDGE DMA-transpose builds the feat-major
    copy used by the linear layers.
  - Hop-1 shards are AllGathered (bf16) into every core's DRAM so hop 2 can
    gather from the full h1.
  - Linear stage runs feat-major: out.T[fo, n] = relu(W.T @ hT + b), bias and
    relu fused in one ScalarE activation (bias is per-partition there).
    The [1536, 1250] per-core outputs are concatenated + transposed on host.
"""

import sys

sys.path.insert(0, "/opt/trn_rl_repo")

import numpy as np
import ml_dtypes

import concourse.bass as bass
import concourse.bacc as bacc
import concourse.mybir as mybir
import concourse.tile as tile
from concourse import library_config
from concourse.bass_utils import run_bass_kernel_spmd

N_NODES = 10000
N_EDGES = 160000
D = 512
ORDER = 2
N_CORES = 8
SHARD = N_NODES // N_CORES          # 1250
BLKS = (SHARD + 127) // 128         # 10 dst blocks per core
BLK_SZ = [min(128, SHARD - b * 128) for b in range(BLKS)]  # [128]*9 + [98]
FI = D // 128                       # 4 feat-in chunks
FO = D // 128                       # 4 feat-out tiles
NGRPS = [512, 512, SHARD - 1024]    # node groups for linear stage
BF16 = ml_dtypes.bfloat16


def _split_excess_waits(nc, max_waits=1):
    """This walrus build rejects >1 sync wait per instruction (and any on a
    Drain). Hoist excess SyncWaits onto InstNoOp carriers inserted just
    before, on the same engine — waits execute in program order, so
    semantics are preserved."""
    for fn in nc.m.functions:
        for bb in fn.blocks:
            new = []
            changed = False
            for inst in bb.instructions:
                si = inst.sync_info
                cap = 0 if isinstance(inst, mybir.InstDrain) else max_waits
                if si is not None and len(si.on_wait) > cap:
                    waits = list(si.on_wait)
                    excess = waits[:-cap] if cap else waits
                    keep = waits[-cap:] if cap else []
                    for g in range(0, len(excess), max_waits):
                        nop = mybir.InstNoOp(name=f"{inst.name}-ws{g}", ins=[], outs=[])
                        nop.engine = inst.engine
                        nop.sync_info = mybir.SyncInfo(
                            on_wait=excess[g:g + max_waits], on_update=[])
                        new.append(nop)
                    si.on_wait = keep
                    changed = True
                new.append(inst)
            if changed:
                bb.instructions = new


def _preprocess(features, D_norm, edge_w, W, b, src, dst):
    """Host-side: shard edges by dst owner, sort, chunk, build S / idx."""
    core_of = dst // SHARD
    per_core = []
    for i in range(N_CORES):
        sel = np.nonzero(core_of == i)[0]
        dl = dst[sel] - i * SHARD          # local dst in [0, SHARD)
        order = np.argsort(dl, kind="stable")
        per_core.append((sel[order], dl[order]))

    # chunk counts per block, maxed over cores so the SPMD program is shared
    nchk = np.zeros(BLKS, np.int64)
    for i in range(N_CORES):
        _, dl = per_core[i]
        blk = dl // 128
        cnt = np.bincount(blk, minlength=BLKS)
        nchk = np.maximum(nchk, (cnt + 127) // 128)
    nchk = np.maximum(nchk, 1).astype(np.int64)
    ncht = int(nchk.sum())

    idx_cols = ncht * 8                     # 16 idx per column
    idx_packed = np.zeros((N_CORES, 128, idx_cols), np.int16)
    s_t = np.zeros((N_CORES, 128, ncht, 128), np.float32)
    blk_chunk_off = np.concatenate([[0], np.cumsum(nchk)])

    for i in range(N_CORES):
        eids, dl = per_core[i]
        w = edge_w[eids]
        s = src[eids]
        blk = dl // 128
        m = dl - blk * 128
        # position of each edge within its block
        cnt = np.bincount(blk, minlength=BLKS)
        boff = np.concatenate([[0], np.cumsum(cnt)])[:-1]
        pos_in_blk = np.arange(len(eids)) - boff[blk]
        chunk = blk_chunk_off[blk] + pos_in_blk // 128
        lane = pos_in_blk % 128
        s_t[i, lane, chunk, m] = w
        lin = chunk * 128 + lane            # linear gather slot
        idx_flat = np.zeros(ncht * 128, np.int16)  # pads gather node 0
        idx_flat[lin] = s.astype(np.int16)
        idx_packed[i] = np.tile(idx_flat.reshape(-1, 16).T, (8, 1))

    return nchk, ncht, idx_packed, s_t.astype(BF16), per_core


def _build_program(nchk, ncht, split_waits=True):
    nc = bacc.Bacc("TRN2", num_swdge_queues=4)
    dt = mybir.dt

    h0_full = nc.declare_dram_parameter("h0_full", [N_NODES, D], dt.bfloat16, isOutput=False)
    h0t_shard = nc.declare_dram_parameter("h0t_shard", [128, FI, SHARD], dt.bfloat16, isOutput=False)
    idx_in = nc.declare_dram_parameter("idx", [128, ncht * 8], dt.int16, isOutput=False)
    s_in = nc.declare_dram_parameter("s", [128, ncht, 128], dt.bfloat16, isOutput=False)
    d_in = nc.declare_dram_parameter("dnorm", [128, BLKS], dt.float32, isOutput=False)
    w_in = nc.declare_dram_parameter("w", [128, ORDER + 1, FI, D], dt.bfloat16, isOutput=False)
    b_in = nc.declare_dram_parameter("bias", [128, ORDER + 1, FO], dt.float32, isOutput=False)
    out_t = nc.declare_dram_parameter("out_t", [(ORDER + 1) * D, SHARD], dt.float32, isOutput=True)

    h1_shard_dram = nc.dram_tensor("h1_shard", [SHARD, D], dt.bfloat16)
    h1_full = nc.dram_tensor("h1_full", [N_NODES, D], dt.bfloat16, addr_space="Shared")

    blk_off = np.concatenate([[0], np.cumsum(nchk)])

    with tile.TileContext(nc) as tc:
        nc.gpsimd.load_library(library_config.mlp)
        with (
            tc.tile_pool(name="const", bufs=1) as const,
            tc.tile_pool(name="gbuf", bufs=3) as gbuf,
            tc.tile_pool(name="evict", bufs=3) as evict,
            tc.tile_pool(name="lin", bufs=3) as lin,
            tc.tile_pool(name="psum", bufs=4, space=bass.MemorySpace.PSUM) as psum,
            tc.tile_pool(name="psw", bufs=4, space=bass.MemorySpace.PSUM) as psw,
        ):
            idx_t = const.tile([128, ncht * 8], dt.int16)
            nc.sync.dma_start(idx_t[:], idx_in[:])
            s_t = const.tile([128, ncht, 128], dt.bfloat16)
            nc.sync.dma_start(s_t[:], s_in[:])
            d_t = const.tile([128, BLKS], dt.float32)
            nc.sync.dma_start(d_t[:], d_in[:])
            w_t = const.tile([128, ORDER + 1, FI, D], dt.bfloat16)
            nc.sync.dma_start(w_t[:], w_in[:])
            b_t = const.tile([128, ORDER + 1, FO], dt.float32)
            nc.sync.dma_start(b_t[:], b_in[:])

            # feat-major hop results; hT[p, f, n] = h[n, f*128+p]
            # free dim padded to BLKS*128 so the last block's transpose fits
            ht = [const.tile([128, FI, BLKS * 128], dt.bfloat16, tag=f"ht{k}",
                             name=f"ht{k}")
                  for k in range(ORDER + 1)]
            nc.sync.dma_start(ht[0][:, :, :SHARD], h0t_shard[:])

            def hop(src_dram, k):
                """One SpMM hop: gather from src_dram, scatter into shard
                blocks, scale by D_norm, emit bf16 node-major blocks +
                feat-major ht[k]."""
                blocks = []
                for bi in range(BLKS):
                    nch = int(nchk[bi])
                    off = int(blk_off[bi])
                    g = gbuf.tile([128, nch, D], dt.bfloat16, tag="g")
                    nc.gpsimd.dma_gather(
                        out_ap=g[:],
                        in_ap=src_dram[:],
                        idxs_ap=idx_t[:, off * 8:(off + nch) * 8],
                        num_idxs=nch * 128,
                        num_idxs_reg=nch * 128,
                        elem_size=D,
                        # >~2000 idxs overflows the single-packet SWDGE ring
                        single_packet=False,
                        queue_num=bi % 4,
                    )
                    sz = BLK_SZ[bi]
                    # full 128 dst columns: S pad columns are zero, so rows
                    # beyond the block size come out as defined zeros
                    acc = psum.tile([128, D], dt.float32, tag="agg")
                    for c in range(nch):
                        nc.tensor.matmul(
                            acc[:], s_t[:, off + c, :], g[:, c, :],
                            start=(c == 0), stop=(c == nch - 1))
                    hb = evict.tile([128, D], dt.bfloat16, tag="hb")
                    nc.scalar.activation(
                        out=hb[:], in_=acc[:],
                        func=mybir.ActivationFunctionType.Copy,
                        scale=d_t[:, bi:bi + 1])
                    # feat-major transpose into ht[k]
                    nc.sync.dma_start_transpose(
                        ht[k][:, :, bi * 128:bi * 128 + 128], hb[:])
                    blocks.append((hb, sz))
                return blocks

            blocks1 = hop(h0_full, 1)
            for bi, (hb, sz) in enumerate(blocks1):
                nc.sync.dma_start(
                    h1_shard_dram[bi * 128:bi * 128 + sz, :], hb[:sz, :])
            nc.gpsimd.collective_compute(
                "AllGather",
                mybir.AluOpType.bypass,
                replica_groups=[list(range(N_CORES))],
                ins=[h1_shard_dram[:]],
                outs=[h1_full[:]],
            )
            hop(h1_full, 2)

            # Linear stage, feat-major: outT[fo, n] = relu(sum_fi W[fi,fo]^T hT + b)
            for k in range(ORDER + 1):
                for ft in range(FO):
                    for gi, gsz in enumerate(NGRPS):
                        goff = sum(NGRPS[:gi])
                        pw = psw.tile([128, gsz], dt.float32, tag="pw")
                        for fi in range(FI):
                            nc.tensor.matmul(
                                pw[:], w_t[:, k, fi, ft * 128:(ft + 1) * 128],
                                ht[k][:, fi, goff:goff + gsz],
                                start=(fi == 0), stop=(fi == FI - 1))
                        ob = lin.tile([128, gsz], dt.float32, tag="ob")
                        nc.scalar.activation(
                            out=ob[:], in_=pw[:],
                            func=mybir.ActivationFunctionType.Relu,
                            bias=b_t[:, k, ft:ft + 1])
                        nc.sync.dma_start(
                            out_t[k * D + ft * 128:k * D + (ft + 1) * 128,
                                  goff:goff + gsz], ob[:])

    nc.compile()
    if split_waits:
        _split_excess_waits(nc)
    return nc


def kernel(features, D_norm, edge_w, W, b, src, dst, _timing=None):
    features = np.asarray(features, np.float32)
    D_norm = np.asarray(D_norm, np.float32)
    edge_w = np.asarray(edge_w, np.float32)
    W = np.asarray(W, np.float32)
    b = np.asarray(b, np.float32)
    src = np.asarray(src, np.int32)
    dst = np.asarray(dst, np.int32)

    nchk, ncht, idx_packed, s_t, _ = _preprocess(
        features, D_norm, edge_w, W, b, src, dst)
    nc = _build_program(nchk, ncht)

    h0_bf = features.astype(BF16)
    # h0t[p, f, n] = h0[shard+n, f*128+p]
    d_pack = np.zeros((128, BLKS), np.float32)
    w_pack = np.zeros((128, ORDER + 1, FI, D), np.float32)
    for fi in range(FI):
        w_pack[:, :, fi, :] = W[:, fi * 128:(fi + 1) * 128, :].transpose(1, 0, 2)
    b_pack = np.zeros((128, ORDER + 1, FO), np.float32)
    for ft in range(FO):
        b_pack[:, :, ft] = b[:, ft * 128:(ft + 1) * 128].T

    in_maps = []
    for i in range(N_CORES):
        sh = slice(i * SHARD, (i + 1) * SHARD)
        h0t = features[sh].reshape(SHARD, FI, 128).transpose(2, 1, 0)
        dp = d_pack.copy()
        dflat = D_norm[sh, 0]
        for bi in range(BLKS):
            dp[:BLK_SZ[bi], bi] = dflat[bi * 128:bi * 128 + BLK_SZ[bi]]
        in_maps.append({
            "h0_full": h0_bf,
            "h0t_shard": h0t.astype(BF16).copy(),
            "idx": idx_packed[i],
            "s": s_t[i],
            "dnorm": dp,
            "w": w_pack.astype(BF16),
            "bias": b_pack,
        })

    res = run_bass_kernel_spmd(
        nc, in_maps, list(range(N_CORES)),
        trace=bool(_timing is not None))
    if _timing is not None:
        _timing["exec_time_ns"] = res.exec_time_ns

    parts = [np.asarray(res.results[i]["out_t"]) for i in range(N_CORES)]
    out = np.concatenate(parts, axis=1).T          # [N, 3*D]
    return np.ascontiguousarray(out, dtype=np.float32)
